# revision 2
# baseline (speedup 1.0000x reference)
"""CostVolumeLayer3D Trainium2 kernel (final).

Product-bound design point: DVE tensor_tensor at ~2 elem/lane/cycle is the
architectural wall (~384us/core), so v5 minimizes DVE instruction count and
keeps every other engine off the critical path.

- Host pre-builds FIVE x-shifted, halo-free x2 variants (xb = 0..4, width
  64, y-stride 64). On device, (y,x) merge into one contiguous AP dim, so
  one TT per (diagonal, y-block) covers all 4 t-slices x 5 d-shifts via an
  overlapping (t, dd) access pattern: 72 TT ops total (vs 180), all
  4B-aligned (no odd-shift copies, no ACT prep).
- One-hot reduction matmuls: [128, 30] lhsT per shift into three 32-aligned
  PSUM col-groups (cheap LDWEIGHTS, col-group overlap), 20 matmuls per TT.
- ACT extracts rows 0..94 with the 1/125 scale; host reassembles the 45
  surviving channels of the 125-channel output.

Sharding: depth D=32 -> 4 output slices per core (8 cores), halo-padded
x2 shards. Partitions = (b, c) = 2*64 = 128.
"""

import numpy as np

_B, _C, _D, _H, _W = 2, 64, 32, 64, 64
_R = 2
_NCH = 125
_RNG = 2 * _R + 1
_NCORES = 8
_DL = _D // _NCORES          # 4
_DH = _DL + 2 * _R           # 8
_YB = 8                      # y-block rows
_YHH = _YB + 2 * _R          # 12
_HP = _H + 2 * _R            # 68 padded y rows
_NG = 3                      # psum col groups
_GW = 15                     # shifts per group
_GROWS = 2 * _GW             # 30


def _shift_table():
    shifts = []
    for sd in range(-4, 5):
        i = min(2, sd + 2)
        j = sd - i
        for h in range(-2, 3):
            shifts.append(((5 * sd + h) % _NCH, _R - h, _R - i, _R - j))
    return shifts


_SHIFTS = _shift_table()
_NS = len(_SHIFTS)           # 45


def _ones_lhst(np_dt):
    a = np.zeros((_NS, 128, _GROWS), dtype=np_dt)
    for s in range(_NS):
        i = s % _GW
        a[s, 0:64, 2 * i] = 1.0
        a[s, 64:128, 2 * i + 1] = 1.0
    return a


_prog = None


def _build_program():
    global _prog
    if _prog is not None:
        return _prog
    from contextlib import ExitStack

    import concourse.bacc as bacc
    import concourse.mybir as mybir
    import concourse.tile as tile
    from concourse.ap import AP

    f16 = mybir.dt.float16
    f32 = mybir.dt.float32
    nc = bacc.Bacc(trn_type="TRN2", debug=False)
    x1_d = nc.dram_tensor("x1", [_B, _C, _DL, _H, _W], f16, kind="ExternalInput")
    # five x-shifted halo-free variants of the padded x2 shard
    x2_d = nc.dram_tensor(
        "x2v", [_RNG, _B, _C, _DH, _HP, _W], f16, kind="ExternalInput"
    )
    on_d = nc.dram_tensor("ones", [_NS, 128, _GROWS], f16, kind="ExternalInput")
    out_d = nc.dram_tensor(
        "out", [_NG, _GW, _B, _DL, _H, _W], f32, kind="ExternalOutput"
    )

    nfree = _YB * _W          # 512
    dstride = _YHH * _W       # variant tile d stride (768 elems)

    with tile.TileContext(nc) as tc:
        with ExitStack() as ctx:
            constp = ctx.enter_context(tc.tile_pool(name="const", bufs=1))
            x2p = ctx.enter_context(tc.tile_pool(name="x2v", bufs=2))
            x1p = ctx.enter_context(tc.tile_pool(name="x1", bufs=2))
            prodp = ctx.enter_context(tc.tile_pool(name="prod", bufs=3))
            psump = ctx.enter_context(tc.tile_pool(name="psum", bufs=2, space="PSUM"))
            stagep = ctx.enter_context(tc.tile_pool(name="stage", bufs=6))

            ones_t = constp.tile([128, _NS, _GROWS], f16)
            nc.sync.dma_start(ones_t[:], on_d.ap().rearrange("s k m -> k s m"))

            for yi in range(_H // _YB):
                y0 = yi * _YB
                x1_t = x1p.tile([128, _DL, _YB, _W], f16, tag="x1")
                nc.sync.dma_start(
                    x1_t[:],
                    x1_d.ap()[:, :, :, y0 : y0 + _YB, :].rearrange(
                        "b c t y x -> (b c) t y x"
                    ),
                )
                x2_t = x2p.tile([128, _RNG, _DH, _YHH, _W], f16, tag="x2v")
                for v in (4, 3, 2, 1, 0):
                    nc.sync.dma_start(
                        x2_t[:, v],
                        x2_d.ap()[v, :, :, :, y0 : y0 + _YHH, :].rearrange(
                            "b c d y x -> (b c) d y x"
                        ),
                    )
                x1_b = x1_t[:].unsqueeze(2).broadcast_to([128, _DL, _RNG, _YB, _W])

                pss = [
                    psump.tile([96, nfree], f32, tag=f"ps{t}", name=f"ps{t}")
                    for t in range(_DL)
                ]
                started = [[False] * _NG for _ in range(_DL)]
                nmm = [[0] * _NG for _ in range(_DL)]
                for di in range(_NS // _RNG):
                    _ch, _dd0, yy0, xx0 = _SHIFTS[_RNG * di]
                    base = x2_t[:, xx0]
                    ov = AP(
                        base.tensor,
                        base.offset + yy0 * _W,
                        [
                            list(base.ap[0]),
                            [dstride, _DL],
                            [dstride, _RNG],
                            [1, nfree],
                        ],
                    )
                    pr = prodp.tile([128, _DL, _RNG, nfree], f16, tag="pr")
                    nc.vector.tensor_mul(
                        pr[:], x1_b.rearrange("p t h y x -> p t h (y x)"), ov
                    )
                    for t in range(_DL):
                        for q in range(_RNG):
                            s = _RNG * di + (_RNG - 1 - q)
                            g = s // _GW
                            nmm[t][g] += 1
                            nc.tensor.matmul(
                                pss[t][32 * g : 32 * g + _GROWS, :],
                                lhsT=ones_t[:, s, :],
                                rhs=pr[:, t, q, :],
                                start=not started[t][g],
                                stop=nmm[t][g] == _GW,
                                tile_position=(0, 32 * g),
                            )
                            started[t][g] = True
                    # group di//3 just completed: stream it out now
                    if di % 3 == 2:
                        g = di // 3
                        for t in range(_DL):
                            st = stagep.tile([_GROWS, nfree], f32, tag="st")
                            nc.scalar.mul(
                                st[:], pss[t][32 * g : 32 * g + _GROWS, :], 1.0 / _NCH
                            )
                            nc.sync.dma_start(
                                out_d.ap()[g, :, :, t, y0 : y0 + _YB, :].rearrange(
                                    "i b y x -> (i b) (y x)"
                                ),
                                st[:],
                            )
    nc.compile()
    _prog = nc
    return nc


def _shard_inputs(x1, x2):
    x2pad = np.pad(
        np.asarray(x2), ((0, 0), (0, 0), (_R, _R), (_R, _R), (_R, _R))
    ).astype(np.float16)
    x1 = np.asarray(x1)
    ones_np = _ones_lhst(np.float16)
    in_maps = []
    for k in range(_NCORES):
        d0 = k * _DL
        shard = x2pad[:, :, d0 : d0 + _DH]           # [B, C, DH, HP, W+4]
        x2v = np.stack(
            [shard[:, :, :, :, xb : xb + _W] for xb in range(_RNG)]
        )                                             # [5, B, C, DH, HP, W]
        in_maps.append(
            {
                "x1": np.ascontiguousarray(x1[:, :, d0 : d0 + _DL].astype(np.float16)),
                "x2v": np.ascontiguousarray(x2v),
                "ones": ones_np,
            }
        )
    return in_maps


def _gather(results):
    out = np.zeros((_B, _NCH, _D, _H, _W), dtype=np.float32)
    for k in range(_NCORES):
        o = results[k]["out"]  # [NG, GW, B, DL, H, W]
        d0 = k * _DL
        for s, (ch, _dd0, _yy0, _xx0) in enumerate(_SHIFTS):
            out[:, ch, d0 : d0 + _DL] = o[s // _GW, s % _GW]
    return out


def _run(in_maps, **kwargs):
    from concourse.bass_utils import run_bass_kernel_spmd

    nc = _build_program()
    return run_bass_kernel_spmd(nc, in_maps, core_ids=list(range(_NCORES)), **kwargs)


def kernel(**inputs):
    res = _run(_shard_inputs(inputs["x1"], inputs["x2"]))
    return _gather(res.results)
